# revision 2
# baseline (speedup 1.0000x reference)
"""LSTM final-h kernel for trn2, 8 NeuronCores, data-parallel over batch.

Per core: 4 sequences. Phase 1 computes xg = x @ W_ih.T + b (tokens t-major)
into DRAM; phase 2 runs the 512-step recurrence with h kept transposed
[128k x (8j*4b)] so each step's matmul is lhsT=h.T tiles, rhs=W_hh.T resident
in SBUF. tanh(z) is computed as 2*sigmoid(2z)-1 (g-gate rows pre-scaled by 2
on host) so one Sigmoid pass covers all gates.
"""
import sys
sys.path.insert(0, '/opt/trn_rl_repo')
import numpy as np

B, T, IN, H = 32, 512, 1024, 1024
G4 = 4 * H  # 4096
NC_ = 8
BL = B // NC_  # 4 per core


def _build():
    import concourse.bass as bass
    import concourse.mybir as mybir
    from concourse import bacc, tile

    f32 = mybir.dt.float32
    nc = bacc.Bacc()

    xT = nc.dram_tensor("xT", [IN, BL * T], f32, kind="ExternalInput")
    wihT = nc.dram_tensor("wihT", [IN, G4], f32, kind="ExternalInput")
    whhT = nc.dram_tensor("whhT", [H, G4], f32, kind="ExternalInput")
    bbc = nc.dram_tensor("bbc", [128, G4], f32, kind="ExternalInput")
    h0T = nc.dram_tensor("h0T", [128, 8 * BL], f32, kind="ExternalInput")
    c0 = nc.dram_tensor("c0", [BL, H], f32, kind="ExternalInput")
    id4 = nc.dram_tensor("id4", [BL, BL], f32, kind="ExternalInput")
    out = nc.dram_tensor("out", [BL, H], f32, kind="ExternalOutput")
    xg = nc.dram_tensor("xg", [BL * T, G4], f32)

    NTOK = BL * T  # 2048
    NTILE = NTOK // 128  # 16

    with tile.TileContext(nc) as tc:
        with (
            tc.tile_pool(name="big", bufs=1) as big,
            tc.tile_pool(name="state", bufs=1) as state,
        ):
            # W region reused: W_ih.T in phase 1, W_hh.T in phase 2.
            W = big.tile([128, 8 * G4], f32)
            hT = state.tile([128, 8 * BL], f32)
            cst = state.tile([BL, H], f32)
            ident = state.tile([BL, BL], f32)

            for j in range(8):
                nc.sync.dma_start(out=W[:, G4 * j:G4 * (j + 1)],
                                  in_=wihT[128 * j:128 * (j + 1), :])
            nc.sync.dma_start(out=hT[:], in_=h0T[:])
            nc.sync.dma_start(out=cst[:], in_=c0[:])
            nc.sync.dma_start(out=ident[:], in_=id4[:])

            # ---- phase 1: xg = x @ W_ih.T + b ----
            with (
                tc.tile_pool(name="p1", bufs=1) as p1,
                tc.tile_pool(name="p1ps", bufs=2, space="PSUM") as p1ps,
            ):
                bb = p1.tile([128, G4], f32, tag="bb")
                nc.sync.dma_start(out=bb[:], in_=bbc[:])
                for n in range(NTILE):
                    xt = p1.tile([128, 8 * 128], f32, tag="xt")
                    for j in range(8):
                        nc.sync.dma_start(
                            out=xt[:, 128 * j:128 * (j + 1)],
                            in_=xT[128 * j:128 * (j + 1), 128 * n:128 * (n + 1)])
                    stage = p1.tile([128, G4], f32, tag="stage")
                    for half in range(2):
                        ps = p1ps.tile([128, 2048], f32)
                        for j in range(8):
                            for q in range(4):
                                col = 2048 * half + 512 * q
                                nc.tensor.matmul(
                                    ps[:, 512 * q:512 * (q + 1)],
                                    xt[:, 128 * j:128 * (j + 1)],
                                    W[:, G4 * j + col:G4 * j + col + 512],
                                    start=(j == 0), stop=(j == 7))
                        nc.vector.tensor_add(
                            stage[:, 2048 * half:2048 * (half + 1)], ps[:],
                            bb[:, 2048 * half:2048 * (half + 1)])
                    nc.sync.dma_start(out=xg[128 * n:128 * (n + 1), :], in_=stage[:])

            # swap in W_hh.T
            for j in range(8):
                nc.sync.dma_start(out=W[:, G4 * j:G4 * (j + 1)],
                                  in_=whhT[128 * j:128 * (j + 1), :])

            # ---- phase 2: recurrence ----
            with (
                tc.tile_pool(name="p2", bufs=1) as p2,
                tc.tile_pool(name="gps", bufs=1, space="PSUM") as gps,
                tc.tile_pool(name="tps", bufs=1, space="PSUM") as tps,
            ):
                with tc.For_i(0, T, 1) as i:
                    xgb = p2.tile([BL, G4], f32, tag="xgb")
                    nc.sync.dma_start(out=xgb[:], in_=xg[bass.ds(i * BL, BL), :])
                    gates = p2.tile([BL, G4], f32, tag="gates")
                    for half in range(2):
                        ps = gps.tile([BL, 2048], f32)
                        for j in range(8):
                            for q in range(4):
                                col = 2048 * half + 512 * q
                                nc.tensor.matmul(
                                    ps[:, 512 * q:512 * (q + 1)],
                                    hT[:, BL * j:BL * (j + 1)],
                                    W[:, G4 * j + col:G4 * j + col + 512],
                                    start=(j == 0), stop=(j == 7))
                        nc.vector.tensor_add(
                            gates[:, 2048 * half:2048 * (half + 1)], ps[:],
                            xgb[:, 2048 * half:2048 * (half + 1)])
                    sig = gates
                    nc.scalar.activation(sig[:], gates[:],
                                         bass.mybir.ActivationFunctionType.Sigmoid)
                    t1 = p2.tile([BL, H], f32, tag="t1")
                    # t1 = i * sig_g ; c = f*c + 2*t1 - i
                    nc.vector.tensor_mul(t1[:], sig[:, 0:H], sig[:, 2 * H:3 * H])
                    nc.vector.tensor_mul(cst[:], cst[:], sig[:, H:2 * H])
                    nc.vector.tensor_add(cst[:], cst[:], t1[:])
                    nc.vector.tensor_add(cst[:], cst[:], t1[:])
                    nc.vector.tensor_sub(cst[:], cst[:], sig[:, 0:H])
                    # h = o * tanh(c) = 2*o*sig(2c) - o
                    s2 = p2.tile([BL, H], f32, tag="s2")
                    nc.scalar.activation(s2[:], cst[:],
                                         bass.mybir.ActivationFunctionType.Sigmoid,
                                         scale=2.0)
                    hh = p2.tile([BL, H], f32, tag="hh")
                    nc.vector.tensor_mul(hh[:], sig[:, 3 * H:4 * H], s2[:])
                    nc.vector.tensor_add(hh[:], hh[:], hh[:])
                    nc.vector.tensor_sub(hh[:], hh[:], sig[:, 3 * H:4 * H])
                    # hT <- transpose(h) via PE, one [128,4] tile per 128-col block
                    tp = tps.tile([128, 8 * BL], f32)
                    for j in range(8):
                        nc.tensor.transpose(tp[:, BL * j:BL * (j + 1)],
                                            hh[:, 128 * j:128 * (j + 1)], ident[:])
                    nc.vector.tensor_copy(hT[:], tp[:])

                nc.sync.dma_start(out=out[:], in_=hh[:])

    nc.finalize()
    return nc


_NC_CACHE = None


def kernel(x, h0, c0, W_ih, W_hh, b_ih, b_hh):
    global _NC_CACHE
    from concourse.bass_utils import run_bass_kernel_spmd

    x = np.asarray(x, np.float32)
    h0 = np.asarray(h0, np.float32)
    c0_ = np.asarray(c0, np.float32)
    W_ih_ = np.asarray(W_ih, np.float32).copy()
    W_hh_ = np.asarray(W_hh, np.float32).copy()
    b = (np.asarray(b_ih, np.float32) + np.asarray(b_hh, np.float32)).copy()
    # pre-scale g-gate rows by 2 for the tanh-via-sigmoid trick
    W_ih_[2 * H:3 * H] *= 2.0
    W_hh_[2 * H:3 * H] *= 2.0
    b[2 * H:3 * H] *= 2.0

    wihT = np.ascontiguousarray(W_ih_.T)
    whhT = np.ascontiguousarray(W_hh_.T)
    bbc = np.ascontiguousarray(np.tile(b[None, :], (128, 1)))
    id4 = np.eye(BL, dtype=np.float32)

    in_maps = []
    for cidx in range(NC_):
        bs = slice(BL * cidx, BL * (cidx + 1))
        xc = x[bs]                                   # [4, 512, 1024]
        xT = np.ascontiguousarray(xc.transpose(2, 1, 0).reshape(IN, T * BL))
        h0c = h0[bs]                                 # [4, 1024]
        h0T = np.ascontiguousarray(h0c.reshape(BL, 8, 128).transpose(2, 1, 0)
                                   .reshape(128, 8 * BL))
        in_maps.append({
            "xT": xT, "wihT": wihT, "whhT": whhT, "bbc": bbc,
            "h0T": h0T, "c0": np.ascontiguousarray(c0_[bs]), "id4": id4,
        })

    if _NC_CACHE is None:
        _NC_CACHE = _build()
    res = run_bass_kernel_spmd(_NC_CACHE, in_maps, list(range(NC_)))
    outs = [np.asarray(res.results[i]["out"]) for i in range(NC_)]
    return np.concatenate(outs, axis=0).astype(np.float32)
